# revision 34
# baseline (speedup 1.0000x reference)
"""Multi-head attention (B=4, S=2048, C=1024, H=16) on 8 TRN2 NeuronCores.

Sharding: data-parallel over batch (4) x query-row split (2); core c handles
batch c//2, query rows [(c%2)*1024, +1024). The host rolls each core's x by
its query-row offset (attention is permutation-invariant over keys) and
passes x^T plus DMA-friendly re-layouts of the weights, all in bfloat16
(matmuls stream at 1 cycle/row in bf16 vs ~2 for fp32; the 2e-2 rel-err
budget easily covers it). No DRAM scratch: Q^T, K^T, V and the attention
output stay resident in SBUF between phases.

  A) QKV projection in bf16. K bias is dropped (softmax is invariant to a
     per-query constant); V bias is folded into the output bias on the host
     (b_eff = b_out + W_out @ b_v, valid because softmax rows sum to 1).
     V lands strided into a [128, st, head, 65] SBUF layout whose 65th lane
     is pre-set to 1.0 - the PV stationary [V_h | 1] then accumulates the
     softmax denominator at out row 64.
  B) Attention per head-pair: transposed scores sc[j,i] = K_h^T(stationary,
     64 PE rows) x Q_h^T(moving); the two heads run row-packed on PE-array
     halves concurrently, into separate per-half PSUM tiles. Head A uses
     2-j-tile PSUM blocks (2 banks) exp'd exactly on the Scalar ACT engine;
     head B uses 1-j-tile blocks (1 bank) exp'd on the Vector engine as a
     one-instruction Schraudolph bf16-exp (u16 = s*EA + EB, bitcast to bf16;
     the ~1.8% RMS sawtooth error cancels its mean in the softmax ratio).
     ACT alone cannot keep up with the PE, so the softmax cost is split
     across both engines. Both halves are double-buffered (2*2 + 2*1 banks)
     plus 2 PV accumulator banks = all 8 PSUM banks, so exp of block t
     overlaps scores of block t+1 with no write-after-read stalls; PV runs
     two j-tiles behind the scores and fills the PE while the exps run.
     Normalization: the PV accumulator is copied out of PSUM immediately
     (freeing the banks), then the tail - 1/denom as exp(-ln d) on ACT
     (the combined ln+exp table is preloaded once so no mid-stream table
     swaps), partition-broadcast on GpSimd, multiply on Vector - is
     deferred into the middle of the NEXT iteration so it never stalls
     the iteration boundary.
  C) Out-projection with O^T stationary; bias (b_eff) added via a
     partition-broadcast tile; the accumulation order puts the
     last-finished OT strip last so Phase C overlaps the final norm tail.

No collectives (a pairwise K/V AllGather dedup was measured a net loss -
see USE_CC); each core writes its own [1024, 1024] fp32 output slice.
"""

from contextlib import ExitStack

import numpy as np
import ml_dtypes

import concourse.mybir as mybir
import concourse.tile as tile
from concourse import bacc
from concourse.bass_utils import run_bass_kernel_spmd

F32 = mybir.dt.float32
BF16 = mybir.dt.bfloat16
U16 = mybir.dt.uint16
AF = mybir.ActivationFunctionType
MUL = mybir.AluOpType.mult
ADD = mybir.AluOpType.add

B, S, C, H, DH = 4, 2048, 1024, 16, 64
NCORES = 8
SCALE = DH ** -0.5  # 0.125
CT = C // 128  # 8 channel tiles
ST = S // 128  # 16 seq (key) tiles
MYROWS = S // 2  # 1024 query rows per core

# Schraudolph bf16-exp: exp(SCALE*s) ~= bitcast_bf16(u16(s*EA + EB)).
# EA = SCALE * 128/ln2; EB = 127*128 - 7.33 (mean-centering) + 0.5 (in case
# the f32->u16 cast truncates; a half-ulp either way is harmless since the
# softmax ratio cancels constant factors).
EXP_EA = SCALE * 128.0 / float(np.log(2.0))
EXP_EB = 16249.17

# Deduplicate the K/V projection across core pairs: each core computes K/V
# only for its own 1024 keys (the x rows it holds at columns 0-1023; the
# per-core roll makes these disjoint within a pair) and the pair exchanges
# blocks with an AllGather. Keys are then used in rank order on both cores -
# attention is permutation-invariant over keys, so no per-core reindexing is
# needed and the SPMD program stays uniform.
# Measured: the AllGather moves 4MB at only ~42GB/s effective in this
# runtime (a ~96us tensor bubble), losing to the ~65us of matmul savings.
# The exchange is correct but a net loss, so it stays disabled.
USE_CC = False




def build():
    nc = bacc.Bacc("TRN2", target_bir_lowering=False, debug=False,
                   num_devices=NCORES)

    # host-prepared layouts (pure data movement on the host):
    #   xT[c, s] = x[s, c]                                   (bf16)
    #   wqk[wt, p, ct*128+f] = W_qkv[wt*128+f, ct*128+p]     (Q/K strips, bf16)
    #   wv[vch, p, ct*512+f] = W_qkv[2C+vch*512+f, ct*128+p] (bf16)
    #   wo[et, p, ct*512+e] = W_out[et*512+e, ct*128+p]      (bf16)
    #   bq2d[p, wt] = b_qkv[wt*128+p] for wt<8               (f32)
    #   beff[e] = b_out[e] + (W_out @ b_v)[e]                (f32)
    xT_in = nc.dram_tensor("xT", [C, S], BF16, kind="ExternalInput").ap()
    wqk_in = nc.dram_tensor("wqk", [16, 128, CT * 128], BF16,
                            kind="ExternalInput").ap()
    wv_in = nc.dram_tensor("wv", [2, 128, CT * 512], BF16,
                           kind="ExternalInput").ap()
    wo_in = nc.dram_tensor("wo", [2, 128, CT * 512], BF16,
                           kind="ExternalInput").ap()
    bq2d = nc.dram_tensor("bq2d", [128, 8], F32, kind="ExternalInput").ap()
    beff = nc.dram_tensor("beff", [C], F32, kind="ExternalInput").ap()
    out = nc.dram_tensor("out", [MYROWS, C], F32, kind="ExternalOutput").ap()
    if USE_CC:
        # staging for the pairwise K/V exchange: cols 0..8191 = K strips
        # (8 x 1024 own keys), cols 8192..16383 = V (8 st x 1024 ch)
        khv_d = nc.dram_tensor("khv_d", [128, 16 * 1024], BF16).ap()
        khv_g = nc.dram_tensor("khv_g", [2, 128, 16 * 1024], BF16).ap()

    with tile.TileContext(nc) as tc, ExitStack() as ctx:
        const = ctx.enter_context(tc.tile_pool(name="const", bufs=1))
        bq_sb = const.tile([128, 8], F32)  # bq_sb[p, wt] = b_q[wt*128+p]
        nc.sync.dma_start(bq_sb[:], bq2d)
        be_sb = const.tile([1, C], F32)
        nc.sync.dma_start(be_sb[:], beff[None, :])
        be_bc = const.tile([128, C], F32)
        nc.gpsimd.partition_broadcast(be_bc[:], be_sb[0:1, :])

        # Pre-load the combined ln+exp activation table (act_func_set_id 6,
        # natural_log_exp_and_others in the cayman act_info.json) so the
        # softmax Exp stream and the 1/denom Ln never force mid-stream
        # ACT_TABLE_LOAD swaps (~1.3us each, 2 per iteration otherwise).
        nc.scalar.add_instruction(mybir.InstLoadActFuncSet(
            name=f"I-{nc.next_id()}", act_func_set_id=6, ins=[], outs=[]))

        persist = ctx.enter_context(tc.tile_pool(name="persist", bufs=1))
        xT = persist.tile([128, CT * S], BF16)  # xT[p, ct*S+s] = x[s, ct*128+p]
        kT = persist.tile([128, 8 * S], BF16)   # kT[p, hp*S+s] = K^T[hp*128+p, s]
        qT = persist.tile([128, 8 * MYROWS], BF16)  # qT[p, hp*1024+i]
        # vt[p, st, h, d] = V[st*128+p, h*64+d] for d<64; 1.0 at d=64
        vt = persist.tile([128, ST * H * 65], BF16)
        vt4 = vt[:].rearrange("p (t h d) -> p t h d", h=H, d=65)
        nc.gpsimd.memset(vt4[:, :, :, 64:65], 1.0)
        OT = persist.tile([128, CT * MYROWS], BF16)  # OT[p, ct*1024+i]

        # ---------------- Phase A: QKV projection ----------------
        with ExitStack() as actx:
            wstrip = actx.enter_context(tc.tile_pool(name="wstrip", bufs=3))
            vw_pool = actx.enter_context(tc.tile_pool(name="vw", bufs=2))
            acc_ps = actx.enter_context(
                tc.tile_pool(name="acc_ps", bufs=4, space="PSUM"))

            # x^T resident: 8 c-tiles of [128, S]
            for ct in range(CT):
                nc.sync.dma_start(xT[:, ct * S:(ct + 1) * S],
                                  xT_in[ct * 128:(ct + 1) * 128, :])

            ksch = 2 if USE_CC else 4  # with CC, only own 1024 keys
            vst = ST // 2 if USE_CC else ST

            # V natural: rhs = W_v^T chunks, output strided into vt (64 of
            # every 65 lanes; lane 64 holds the pre-set ones column)
            vws = []
            for vch in range(2):
                vw = vw_pool.tile([128, CT * 512], BF16)
                nc.sync.dma_start(vw[:], wv_in[vch])
                vws.append(vw)
            for st in range(vst):
                for vch in range(2):
                    acc = acc_ps.tile([128, 512], F32)
                    for ct in range(CT):
                        nc.tensor.matmul(
                            acc[:],
                            xT[:, ct * S + st * 128: ct * S + (st + 1) * 128],
                            vws[vch][:, ct * 512:(ct + 1) * 512],
                            start=(ct == 0), stop=(ct == CT - 1))
                    nc.vector.tensor_copy(
                        vt4[:, st, vch * 8:(vch + 1) * 8, 0:64],
                        acc[:].rearrange("p (h d) -> p h d", d=64))
                if USE_CC:
                    nc.sync.dma_start(
                        khv_d[:, 8192 + st * 1024: 8192 + (st + 1) * 1024]
                        .rearrange("p (h d) -> p h d", d=64),
                        vt4[:, st, :, 0:64])

            # K^T strips (after V so the first matmul only waits on xT+wv).
            # K needs no bias.
            for wt in range(8, 16):
                ws = wstrip.tile([128, CT * 128], BF16)
                nc.sync.dma_start(ws[:], wqk_in[wt])
                for sch in range(ksch):
                    acc = acc_ps.tile([128, 512], F32)
                    for ct in range(CT):
                        nc.tensor.matmul(
                            acc[:],
                            ws[:, ct * 128:(ct + 1) * 128],
                            xT[:, ct * S + sch * 512: ct * S + sch * 512 + 512],
                            start=(ct == 0), stop=(ct == CT - 1))
                    nc.vector.tensor_copy(
                        kT[:, (wt - 8) * S + sch * 512:
                           (wt - 8) * S + (sch + 1) * 512],
                        acc[:])
                if USE_CC:
                    nc.sync.dma_start(
                        khv_d[:, (wt - 8) * 1024:(wt - 7) * 1024],
                        kT[:, (wt - 8) * S:(wt - 8) * S + 1024])

            if USE_CC:
                nc.gpsimd.collective_compute(
                    "AllGather", mybir.AluOpType.bypass,
                    [[0, 1], [2, 3], [4, 5], [6, 7]],
                    ins=[khv_d], outs=[khv_g])

            # Q^T strips (covers the exchange latency when USE_CC)
            for wt in range(8):
                ws = wstrip.tile([128, CT * 128], BF16)
                nc.sync.dma_start(ws[:], wqk_in[wt])
                for sch in range(2):
                    acc = acc_ps.tile([128, 512], F32)
                    for ct in range(CT):
                        nc.tensor.matmul(
                            acc[:],
                            ws[:, ct * 128:(ct + 1) * 128],
                            xT[:, ct * S + sch * 512: ct * S + sch * 512 + 512],
                            start=(ct == 0), stop=(ct == CT - 1))
                    nc.vector.tensor_scalar_add(
                        qT[:, wt * MYROWS + sch * 512:
                           wt * MYROWS + (sch + 1) * 512],
                        acc[:], bq_sb[:, wt:wt + 1])

            if USE_CC:
                # scatter the gathered blocks: keys live in RANK order on
                # every core (rank r's block -> key cols r*1024, st r*8).
                # The own-rank block is rewritten in place with identical
                # data, which keeps the SPMD program uniform.
                for r in range(2):
                    for wt in range(8):
                        nc.sync.dma_start(
                            kT[:, wt * S + r * 1024: wt * S + (r + 1) * 1024],
                            khv_g[r, :, wt * 1024:(wt + 1) * 1024])
                    for stl in range(8):
                        nc.sync.dma_start(
                            vt4[:, r * 8 + stl, :, 0:64],
                            khv_g[r, :, 8192 + stl * 1024:
                                  8192 + (stl + 1) * 1024]
                            .rearrange("p (h d) -> p h d", d=64))

        # Prefetch the out-projection weights during Phase B so Phase C
        # doesn't stall on their DMA.
        woTs = []
        for et in range(2):
            woT = persist.tile([128, CT * 512], BF16)  # [c_p, ct, 512 e]
            nc.sync.dma_start(woT[:], wo_in[et])
            woTs.append(woT)

        # ---------------- Phase B: attention ----------------
        with ExitStack() as bctx:
            pp = bctx.enter_context(tc.tile_pool(name="pp", bufs=5))
            smalls = bctx.enter_context(tc.tile_pool(name="smalls", bufs=5))
            sc_ps = bctx.enter_context(
                tc.tile_pool(name="sc_ps", bufs=1, space="PSUM"))
            pv_ps = bctx.enter_context(
                tc.tile_pool(name="pv_ps", bufs=1, space="PSUM"))

            def emit_norm(uvs, hp, ich):
                # deferred normalization tail (reads SBUF only): 1/denom as
                # exp(-ln d) on ACT (vector.reciprocal is ~3.3us/call and
                # the approx-fast custom op is broken in this runtime),
                # broadcast on GpSimd, multiply on Vector.
                for half in range(2):
                    uv = uvs[half]
                    lnt = smalls.tile([1, 512], F32)
                    nc.scalar.activation(lnt[:], uv[64:65, :], AF.Ln)
                    rec = smalls.tile([1, 512], F32)
                    nc.scalar.activation(rec[:], lnt[:], AF.Exp, scale=-1.0)
                    rb = smalls.tile([64, 512], F32)
                    nc.gpsimd.partition_broadcast(rb[:], rec[0:1, :])
                    nc.gpsimd.tensor_mul(
                        OT[half * 64:half * 64 + 64,
                           hp * MYROWS + ich * 512:
                           hp * MYROWS + (ich + 1) * 512],
                        uv[0:64, :], rb[:])

            pending = []  # deferred (uvs, hp, ich) normalization tails
            # hp=7 first: Phase C's accumulation is ordered to END on hp6's
            # OT strip, so C's first matmuls overlap the final norm tail.
            # Both 512-query chunks (ich) run in ONE j loop: each kt/vt
            # stationary is loaded once and reused for both chunks, halving
            # the exposed weight-buffer loads (the concurrent score pair
            # occupies both PE weight buffers, so un-reused loads cannot
            # prefetch and cost ~100ns each).
            for hp in [7] + list(range(7)):  # A = rows 0-63, B = 64-127
                kt = kT[:, hp * S:(hp + 1) * S]
                qts = [qT[:, hp * MYROWS + ich * 512:
                          hp * MYROWS + (ich + 1) * 512] for ich in range(2)]
                pvs = [[pv_ps.tile([128, 512], F32, tag=f"pv{ich}{half}",
                                   name=f"pv{ich}{half}")
                        for half in range(2)] for ich in range(2)]
                # pg[half][j] = exp'd scores [128, 1024] (q-chunks side by side)
                pgj = [[None] * ST, [None] * ST]

                def emit_pv(j):
                    for half in range(2):
                        for ich in range(2):
                            nc.tensor.matmul(
                                pvs[ich][half][0:65, :],
                                vt4[:, j, 2 * hp + half, 0:65],
                                pgj[half][j][:, ich * 512:(ich + 1) * 512],
                                start=(j == 0), stop=(j == ST - 1))

                for j in range(ST):
                    sca = sc_ps.tile([128, 1024], F32, tag="scA", name="scA")
                    scb = sc_ps.tile([128, 1024], F32, tag="scB", name="scB")
                    # order A0,B0,B1,A1: each half's stationary stays in its
                    # weight buffer for both chunks; row-disjoint A/B overlap
                    nc.tensor.matmul(sca[:, 0:512],
                                     kt[0:64, j * 128:(j + 1) * 128],
                                     qts[0][0:64, :], start=True, stop=True)
                    nc.tensor.matmul(scb[:, 0:512],
                                     kt[64:128, j * 128:(j + 1) * 128],
                                     qts[0][64:128, :], start=True, stop=True)
                    nc.tensor.matmul(scb[:, 512:1024],
                                     kt[64:128, j * 128:(j + 1) * 128],
                                     qts[1][64:128, :], start=True, stop=True)
                    nc.tensor.matmul(sca[:, 512:1024],
                                     kt[0:64, j * 128:(j + 1) * 128],
                                     qts[1][0:64, :], start=True, stop=True)
                    # half A exact on ACT, half B Schraudolph on Vector,
                    # each as one [128, 1024] instruction
                    pga = pp.tile([128, 1024], BF16, tag="pga", name="pga")
                    nc.scalar.activation(pga[:], sca[:], AF.Exp, scale=SCALE)
                    pgb = pp.tile([128, 1024], BF16, tag="pgb", name="pgb")
                    nc.vector.tensor_scalar(pgb[:].bitcast(U16), scb[:],
                                            EXP_EA, EXP_EB, MUL, ADD)
                    pgj[0][j] = pga[:]
                    pgj[1][j] = pgb[:]
                    # software pipeline: PV trails the scores by 3 j
                    if j >= 3:
                        emit_pv(j - 3)
                    # previous head-pair's deferred norm tails, mid-loop
                    if j in (6, 10) and pending:
                        emit_norm(*pending.pop(0))
                emit_pv(ST - 3)
                emit_pv(ST - 2)
                emit_pv(ST - 1)

                # copy [out|denom] rows out of PSUM immediately so the
                # accumulator banks free; the norm tails are deferred
                for ich in range(2):
                    uvs = []
                    for half in range(2):
                        uv = smalls.tile([65, 512], F32)
                        nc.vector.tensor_copy(uv[:], pvs[ich][half][0:65, :])
                        uvs.append(uv)
                    pending.append((uvs, hp, ich))
            for p in pending:
                emit_norm(*p)

        # ---------------- Phase C: out projection ----------------
        with ExitStack() as cctx:
            yt_pool = cctx.enter_context(tc.tile_pool(name="yt", bufs=3))
            y_ps = cctx.enter_context(
                tc.tile_pool(name="y_ps", bufs=2, space="PSUM"))

            for et in range(2):
                woT = woTs[et]
                for it in range(8):
                    y = y_ps.tile([128, 512], F32)
                    for ci, ct in enumerate([7] + list(range(7))):
                        nc.tensor.matmul(
                            y[:],
                            OT[:, ct * MYROWS + it * 128:
                               ct * MYROWS + (it + 1) * 128],
                            woT[:, ct * 512:(ct + 1) * 512],
                            start=(ci == 0), stop=(ci == CT - 1))
                    yt = yt_pool.tile([128, 512], F32)
                    nc.vector.tensor_add(yt[:], y[:],
                                         be_bc[:, et * 512:(et + 1) * 512])
                    nc.sync.dma_start(
                        out[it * 128:(it + 1) * 128,
                            et * 512:(et + 1) * 512], yt[:])

    nc.compile()
    return nc


_cache = {}


def _get_nc():
    if "nc" not in _cache:
        _cache["nc"] = build()
    return _cache["nc"]


def kernel(x_q, W_qkv, b_qkv, W_out, b_out):
    """Core c of 8 handles batch c//2, query rows [(c%2)*1024, +1024).

    The per-core x slice is ROLLED by the core's query-row offset so every
    core's own query rows sit at rows [0, MYROWS) of its slice. Attention is
    permutation-invariant over keys, so the rolled K/V ordering does not
    change the output.
    """
    x_q = np.ascontiguousarray(x_q, dtype=np.float32)
    W_qkv = np.ascontiguousarray(W_qkv, dtype=np.float32)
    b_qkv = np.ascontiguousarray(b_qkv, dtype=np.float32)
    W_out = np.ascontiguousarray(W_out, dtype=np.float32)
    b_out = np.ascontiguousarray(b_out, dtype=np.float32)

    nc = _get_nc()
    in_maps = build_in_maps(x_q, W_qkv, b_qkv, W_out, b_out)
    res = run_bass_kernel_spmd(nc, in_maps, list(range(NCORES)))
    out = np.empty((B, S, C), dtype=np.float32)
    for c in range(NCORES):
        b, half = c // 2, c % 2
        out[b, half * MYROWS:(half + 1) * MYROWS] = res.results[c]["out"]
    return out


def build_in_maps(x_q, W_qkv, b_qkv, W_out, b_out):
    BF = ml_dtypes.bfloat16
    x_q = np.ascontiguousarray(x_q, dtype=np.float32)
    W_qkv = np.asarray(W_qkv, dtype=np.float32)
    b_qkv = np.ascontiguousarray(b_qkv, dtype=np.float32)
    W_out = np.asarray(W_out, dtype=np.float32)
    b_out = np.ascontiguousarray(b_out, dtype=np.float32)
    # wqk[wt, p, ct*128+f] = W_qkv[wt*128+f, ct*128+p]  (Q/K rows only)
    w4 = W_qkv.reshape(24, 128, CT, 128)            # [wt, f, ct, p]
    wqk = np.ascontiguousarray(
        w4[:16].transpose(0, 3, 2, 1).reshape(16, 128, CT * 128).astype(BF))
    # wv[vch, p, ct*512+f] = W_qkv[2C+vch*512+f, ct*128+p]
    wv5 = W_qkv[2 * C:].reshape(2, 512, CT, 128)    # [vch, f, ct, p]
    wv = np.ascontiguousarray(
        wv5.transpose(0, 3, 2, 1).reshape(2, 128, CT * 512).astype(BF))
    # wo[et, p, ct*512+e] = W_out[et*512+e, ct*128+p]
    wo5 = W_out.reshape(2, 512, CT, 128)            # [et, e, ct, p]
    wo = np.ascontiguousarray(
        wo5.transpose(0, 3, 2, 1).reshape(2, 128, CT * 512).astype(BF))
    bq2d = np.ascontiguousarray(b_qkv[:C].reshape(8, 128).T)
    # V bias folded into the output bias (softmax rows sum to 1):
    # out = PV @ W_out.T + (b_v @ W_out.T + b_out)
    beff = np.ascontiguousarray(b_out + W_out @ b_qkv[2 * C:])
    in_maps = []
    for c in range(NCORES):
        b, half = c // 2, c % 2
        xb = x_q[b]
        if half:
            xb = np.roll(xb, -MYROWS, axis=0)
        in_maps.append({
            "xT": np.ascontiguousarray(xb.T.astype(BF)),
            "wqk": wqk,
            "wv": wv,
            "wo": wo,
            "bq2d": bq2d,
            "beff": beff,
        })
    return in_maps


if __name__ == "__main__":
    # smoke test with random inputs
    rng = np.random.default_rng(0)
    x_q = rng.standard_normal((B, S, C), dtype=np.float32)
    s = 1.0 / np.sqrt(C)
    W_qkv = rng.uniform(-s, s, (3 * C, C)).astype(np.float32)
    b_qkv = rng.uniform(-s, s, 3 * C).astype(np.float32)
    W_out = rng.uniform(-s, s, (C, C)).astype(np.float32)
    b_out = rng.uniform(-s, s, C).astype(np.float32)
    got = kernel(x_q=x_q, W_qkv=W_qkv, b_qkv=b_qkv, W_out=W_out, b_out=b_out)
    print("smoke ok", got.shape, float(np.abs(got).max()))


# revision 36
# speedup vs baseline: 1.4120x; 1.4120x over previous
"""Multi-head attention (B=4, S=2048, C=1024, H=16) on 8 TRN2 NeuronCores.

Sharding: data-parallel over batch (4) x query-row split (2); core c handles
batch c//2, query rows [(c%2)*1024, +1024). The host rolls each core's x by
its query-row offset (attention is permutation-invariant over keys) and
passes x^T plus DMA-friendly re-layouts of the weights, all in bfloat16
(matmuls stream at 1 cycle/row in bf16 vs ~2 for fp32; the 2e-2 rel-err
budget easily covers it). No DRAM scratch: Q^T, K^T, V and the attention
output stay resident in SBUF between phases.

  A) QKV projection in bf16. K bias is dropped (softmax is invariant to a
     per-query constant); V bias is folded into the output bias on the host
     (b_eff = b_out + W_out @ b_v, valid because softmax rows sum to 1).
     V lands strided into a [128, st, head, 65] SBUF layout whose 65th lane
     is pre-set to 1.0 - the PV stationary [V_h | 1] then accumulates the
     softmax denominator at out row 64.
  B) Attention per head-pair: transposed scores sc[j,i] = K_h^T(stationary,
     64 PE rows) x Q_h^T(moving); the two heads run row-packed on PE-array
     halves concurrently, into separate per-half PSUM tiles. Head A uses
     2-j-tile PSUM blocks (2 banks) exp'd exactly on the Scalar ACT engine;
     head B uses 1-j-tile blocks (1 bank) exp'd on the Vector engine as a
     one-instruction Schraudolph bf16-exp (u16 = s*EA + EB, bitcast to bf16;
     the ~1.8% RMS sawtooth error cancels its mean in the softmax ratio).
     ACT alone cannot keep up with the PE, so the softmax cost is split
     across both engines. Both halves are double-buffered (2*2 + 2*1 banks)
     plus 2 PV accumulator banks = all 8 PSUM banks, so exp of block t
     overlaps scores of block t+1 with no write-after-read stalls; PV runs
     two j-tiles behind the scores and fills the PE while the exps run.
     Normalization: the PV accumulator is copied out of PSUM immediately
     (freeing the banks), then the tail - 1/denom as exp(-ln d) on ACT
     (the combined ln+exp table is preloaded once so no mid-stream table
     swaps), partition-broadcast on GpSimd, multiply on Vector - is
     deferred into the middle of the NEXT iteration so it never stalls
     the iteration boundary.
  C) Out-projection with O^T stationary; bias (b_eff) added via a
     partition-broadcast tile; the accumulation order puts the
     last-finished OT strip last so Phase C overlaps the final norm tail.

No collectives (a pairwise K/V AllGather dedup was measured a net loss -
see USE_CC); each core writes its own [1024, 1024] fp32 output slice.
"""

from contextlib import ExitStack

import numpy as np
import ml_dtypes

import concourse.mybir as mybir
import concourse.tile as tile
from concourse import bacc
from concourse.bass_utils import run_bass_kernel_spmd

F32 = mybir.dt.float32
BF16 = mybir.dt.bfloat16
U16 = mybir.dt.uint16
AF = mybir.ActivationFunctionType
MUL = mybir.AluOpType.mult
ADD = mybir.AluOpType.add

B, S, C, H, DH = 4, 2048, 1024, 16, 64
NCORES = 8
SCALE = DH ** -0.5  # 0.125
CT = C // 128  # 8 channel tiles
ST = S // 128  # 16 seq (key) tiles
MYROWS = S // 2  # 1024 query rows per core

# Schraudolph bf16-exp: exp(SCALE*s) ~= bitcast_bf16(u16(s*EA + EB)).
# EA = SCALE * 128/ln2; EB = 127*128 - 7.33 (mean-centering) + 0.5 (in case
# the f32->u16 cast truncates; a half-ulp either way is harmless since the
# softmax ratio cancels constant factors).
EXP_EA = SCALE * 128.0 / float(np.log(2.0))
EXP_EB = 16249.17

# Deduplicate the K/V projection across core pairs: each core computes K/V
# only for its own 1024 keys (the x rows it holds at columns 0-1023; the
# per-core roll makes these disjoint within a pair) and the pair exchanges
# blocks with an AllGather. Keys are then used in rank order on both cores -
# attention is permutation-invariant over keys, so no per-core reindexing is
# needed and the SPMD program stays uniform.
# Measured: the AllGather moves 4MB at only ~42GB/s effective in this
# runtime (a ~96us tensor bubble), losing to the ~65us of matmul savings.
# The exchange is correct but a net loss, so it stays disabled.
USE_CC = False




def build():
    nc = bacc.Bacc("TRN2", target_bir_lowering=False, debug=False,
                   num_devices=NCORES)

    # host-prepared layouts (pure data movement on the host):
    #   xT[c, s] = x[s, c]                                   (bf16)
    #   wqk[wt, p, ct*128+f] = W_qkv[wt*128+f, ct*128+p]     (Q/K strips, bf16)
    #   wv[vch, p, ct*512+f] = W_qkv[2C+vch*512+f, ct*128+p] (bf16)
    #   wo[et, p, ct*512+e] = W_out[et*512+e, ct*128+p]      (bf16)
    #   bq2d[p, wt] = b_qkv[wt*128+p] for wt<8               (f32)
    #   beff[e] = b_out[e] + (W_out @ b_v)[e]                (f32)
    xT_in = nc.dram_tensor("xT", [C, S], BF16, kind="ExternalInput").ap()
    wqk_in = nc.dram_tensor("wqk", [16, 128, CT * 128], BF16,
                            kind="ExternalInput").ap()
    wv_in = nc.dram_tensor("wv", [2, 128, CT * 512], BF16,
                           kind="ExternalInput").ap()
    wo_in = nc.dram_tensor("wo", [2, 128, CT * 512], BF16,
                           kind="ExternalInput").ap()
    bq2d = nc.dram_tensor("bq2d", [128, 8], F32, kind="ExternalInput").ap()
    beff = nc.dram_tensor("beff", [C], F32, kind="ExternalInput").ap()
    out = nc.dram_tensor("out", [MYROWS, C], F32, kind="ExternalOutput").ap()
    if USE_CC:
        # staging for the pairwise K/V exchange: cols 0..8191 = K strips
        # (8 x 1024 own keys), cols 8192..16383 = V (8 st x 1024 ch)
        khv_d = nc.dram_tensor("khv_d", [128, 16 * 1024], BF16).ap()
        khv_g = nc.dram_tensor("khv_g", [2, 128, 16 * 1024], BF16).ap()

    with tile.TileContext(nc) as tc, ExitStack() as ctx:
        const = ctx.enter_context(tc.tile_pool(name="const", bufs=1))
        bq_sb = const.tile([128, 8], F32)  # bq_sb[p, wt] = b_q[wt*128+p]
        nc.sync.dma_start(bq_sb[:], bq2d)
        be_sb = const.tile([1, C], F32)
        nc.sync.dma_start(be_sb[:], beff[None, :])
        be_bc = const.tile([128, C], F32)
        nc.gpsimd.partition_broadcast(be_bc[:], be_sb[0:1, :])

        # Pre-load the combined ln+exp activation table (act_func_set_id 6,
        # natural_log_exp_and_others in the cayman act_info.json) so the
        # softmax Exp stream and the 1/denom Ln never force mid-stream
        # ACT_TABLE_LOAD swaps (~1.3us each, 2 per iteration otherwise).
        nc.scalar.add_instruction(mybir.InstLoadActFuncSet(
            name=f"I-{nc.next_id()}", act_func_set_id=6, ins=[], outs=[]))

        persist = ctx.enter_context(tc.tile_pool(name="persist", bufs=1))
        xT = persist.tile([128, CT * S], BF16)  # xT[p, ct*S+s] = x[s, ct*128+p]
        kT = persist.tile([128, 8 * S], BF16)   # kT[p, hp*S+s] = K^T[hp*128+p, s]
        qT = persist.tile([128, 8 * MYROWS], BF16)  # qT[p, hp*1024+i]
        # vt[p, st, h, d] = V[st*128+p, h*64+d] for d<64; 1.0 at d=64
        vt = persist.tile([128, ST * H * 65], BF16)
        vt4 = vt[:].rearrange("p (t h d) -> p t h d", h=H, d=65)
        nc.gpsimd.memset(vt4[:, :, :, 64:65], 1.0)
        OT = persist.tile([128, CT * MYROWS], BF16)  # OT[p, ct*1024+i]

        # ---------------- Phase A: QKV projection ----------------
        with ExitStack() as actx:
            wstrip = actx.enter_context(tc.tile_pool(name="wstrip", bufs=3))
            vw_pool = actx.enter_context(tc.tile_pool(name="vw", bufs=2))
            acc_ps = actx.enter_context(
                tc.tile_pool(name="acc_ps", bufs=4, space="PSUM"))

            # x^T resident: 8 c-tiles of [128, S]
            for ct in range(CT):
                nc.sync.dma_start(xT[:, ct * S:(ct + 1) * S],
                                  xT_in[ct * 128:(ct + 1) * 128, :])

            ksch = 2 if USE_CC else 4  # with CC, only own 1024 keys
            vst = ST // 2 if USE_CC else ST

            # V natural: rhs = W_v^T chunks, output strided into vt (64 of
            # every 65 lanes; lane 64 holds the pre-set ones column)
            vws = []
            for vch in range(2):
                vw = vw_pool.tile([128, CT * 512], BF16)
                nc.sync.dma_start(vw[:], wv_in[vch])
                vws.append(vw)
            for st in range(vst):
                for vch in range(2):
                    acc = acc_ps.tile([128, 512], F32)
                    for ct in range(CT):
                        nc.tensor.matmul(
                            acc[:],
                            xT[:, ct * S + st * 128: ct * S + (st + 1) * 128],
                            vws[vch][:, ct * 512:(ct + 1) * 512],
                            start=(ct == 0), stop=(ct == CT - 1))
                    nc.vector.tensor_copy(
                        vt4[:, st, vch * 8:(vch + 1) * 8, 0:64],
                        acc[:].rearrange("p (h d) -> p h d", d=64))
                if USE_CC:
                    nc.sync.dma_start(
                        khv_d[:, 8192 + st * 1024: 8192 + (st + 1) * 1024]
                        .rearrange("p (h d) -> p h d", d=64),
                        vt4[:, st, :, 0:64])

            # K^T strips (after V so the first matmul only waits on xT+wv).
            # K needs no bias.
            for wt in range(8, 16):
                ws = wstrip.tile([128, CT * 128], BF16)
                nc.sync.dma_start(ws[:], wqk_in[wt])
                for sch in range(ksch):
                    acc = acc_ps.tile([128, 512], F32)
                    for ct in range(CT):
                        nc.tensor.matmul(
                            acc[:],
                            ws[:, ct * 128:(ct + 1) * 128],
                            xT[:, ct * S + sch * 512: ct * S + sch * 512 + 512],
                            start=(ct == 0), stop=(ct == CT - 1))
                    nc.vector.tensor_copy(
                        kT[:, (wt - 8) * S + sch * 512:
                           (wt - 8) * S + (sch + 1) * 512],
                        acc[:])
                if USE_CC:
                    nc.sync.dma_start(
                        khv_d[:, (wt - 8) * 1024:(wt - 7) * 1024],
                        kT[:, (wt - 8) * S:(wt - 8) * S + 1024])

            if USE_CC:
                nc.gpsimd.collective_compute(
                    "AllGather", mybir.AluOpType.bypass,
                    [[0, 1], [2, 3], [4, 5], [6, 7]],
                    ins=[khv_d], outs=[khv_g])

            # Q^T strips (covers the exchange latency when USE_CC)
            for wt in range(8):
                ws = wstrip.tile([128, CT * 128], BF16)
                nc.sync.dma_start(ws[:], wqk_in[wt])
                for sch in range(2):
                    acc = acc_ps.tile([128, 512], F32)
                    for ct in range(CT):
                        nc.tensor.matmul(
                            acc[:],
                            ws[:, ct * 128:(ct + 1) * 128],
                            xT[:, ct * S + sch * 512: ct * S + sch * 512 + 512],
                            start=(ct == 0), stop=(ct == CT - 1))
                    nc.vector.tensor_scalar_add(
                        qT[:, wt * MYROWS + sch * 512:
                           wt * MYROWS + (sch + 1) * 512],
                        acc[:], bq_sb[:, wt:wt + 1])

            if USE_CC:
                # scatter the gathered blocks: keys live in RANK order on
                # every core (rank r's block -> key cols r*1024, st r*8).
                # The own-rank block is rewritten in place with identical
                # data, which keeps the SPMD program uniform.
                for r in range(2):
                    for wt in range(8):
                        nc.sync.dma_start(
                            kT[:, wt * S + r * 1024: wt * S + (r + 1) * 1024],
                            khv_g[r, :, wt * 1024:(wt + 1) * 1024])
                    for stl in range(8):
                        nc.sync.dma_start(
                            vt4[:, r * 8 + stl, :, 0:64],
                            khv_g[r, :, 8192 + stl * 1024:
                                  8192 + (stl + 1) * 1024]
                            .rearrange("p (h d) -> p h d", d=64))

        # Prefetch the out-projection weights during Phase B so Phase C
        # doesn't stall on their DMA.
        woTs = []
        for et in range(2):
            woT = persist.tile([128, CT * 512], BF16)  # [c_p, ct, 512 e]
            nc.sync.dma_start(woT[:], wo_in[et])
            woTs.append(woT)

        # ---------------- Phase B: attention ----------------
        with ExitStack() as bctx:
            pp = bctx.enter_context(tc.tile_pool(name="pp", bufs=5))
            smalls = bctx.enter_context(tc.tile_pool(name="smalls", bufs=6))
            sc_ps = bctx.enter_context(
                tc.tile_pool(name="sc_ps", bufs=2, space="PSUM"))
            pv_ps = bctx.enter_context(
                tc.tile_pool(name="pv_ps", bufs=1, space="PSUM"))

            def emit_norm(uvs, hp, ich):
                # deferred normalization tail (reads SBUF only): 1/denom as
                # exp(-ln d) on ACT (vector.reciprocal is ~3.3us/call and
                # the approx-fast custom op is broken in this runtime),
                # broadcast on GpSimd, multiply on Vector.
                for half in range(2):
                    uv = uvs[half]
                    lnt = smalls.tile([1, 512], F32)
                    nc.scalar.activation(lnt[:], uv[64:65, :], AF.Ln)
                    rec = smalls.tile([1, 512], F32)
                    nc.scalar.activation(rec[:], lnt[:], AF.Exp, scale=-1.0)
                    rb = smalls.tile([64, 512], F32)
                    nc.gpsimd.partition_broadcast(rb[:], rec[0:1, :])
                    nc.vector.tensor_mul(
                        OT[half * 64:half * 64 + 64,
                           hp * MYROWS + ich * 512:
                           hp * MYROWS + (ich + 1) * 512],
                        uv[0:64, :], rb[:])

            pending = None  # (uvs, hp, ich) whose norm tail is deferred
            # hp=7 first: Phase C's accumulation is ordered to END on hp6's
            # OT strip, so C's first matmuls overlap the final norm tail
            for hp in [7] + list(range(7)):  # A = rows 0-63, B = 64-127
                kt = kT[:, hp * S:(hp + 1) * S]
                for ich in range(2):
                    qt = qT[:, hp * MYROWS + ich * 512:
                            hp * MYROWS + (ich + 1) * 512]
                    pvs = [pv_ps.tile([128, 512], F32, tag=f"pv{half}",
                                      name=f"pv{half}")
                           for half in range(2)]
                    # pg[j] = (ap of exp'd scores for j, per half)
                    pgj = [[None] * ST, [None] * ST]

                    def emit_pv(j, halves=(0, 1)):
                        for half in halves:
                            nc.tensor.matmul(
                                pvs[half][0:65, :],
                                vt4[:, j, 2 * hp + half, 0:65],
                                pgj[half][j],
                                start=(j == 0), stop=(j == ST - 1))

                    sca = None
                    for j in range(ST):
                        jj = j % 2
                        if jj == 0:
                            sca = sc_ps.tile([128, 1024], F32, tag="scA",
                                             name="scA")
                        scb = sc_ps.tile([128, 512], F32, tag="scB",
                                         name="scB")
                        # interleave a row-full PV matmul between the two
                        # score matmuls: every LDWEIGHTS then has a free PE
                        # weight buffer to prefetch into (the concurrent A|B
                        # pair otherwise holds both buffers and exposes the
                        # following PV weight loads, ~230ns/j)
                        nc.tensor.matmul(sca[:, jj * 512:(jj + 1) * 512],
                                         kt[0:64, j * 128:(j + 1) * 128],
                                         qt[0:64, :], start=True, stop=True)
                        if j >= 3:
                            emit_pv(j - 3, halves=(0,))
                        nc.tensor.matmul(scb[:],
                                         kt[64:128, j * 128:(j + 1) * 128],
                                         qt[64:128, :], start=True, stop=True)
                        if j >= 3:
                            emit_pv(j - 3, halves=(1,))
                        # half B: Schraudolph bf16-exp on the Vector engine
                        # (last two j on ACT instead - DVE is the busier
                        # engine, and this also trims the approx fraction)
                        pgb = pp.tile([128, 512], BF16, tag="pgb", name="pgb")
                        if j < 14:
                            nc.vector.tensor_scalar(
                                pgb[:].bitcast(U16), scb[:],
                                EXP_EA, EXP_EB, MUL, ADD)
                        else:
                            nc.scalar.activation(pgb[:], scb[:],
                                                 AF.Exp, scale=SCALE)
                        pgj[1][j] = pgb[:]
                        # half A: exact exp on the ACT engine, per 2-j block
                        if jj == 1:
                            pga = pp.tile([128, 1024], BF16, tag="pga",
                                          name="pga")
                            nc.scalar.activation(pga[:], sca[:],
                                                 AF.Exp, scale=SCALE)
                            pgj[0][j - 1] = pga[:, 0:512]
                            pgj[0][j] = pga[:, 512:1024]
                        # previous iteration's normalization tail runs here,
                        # mid-iteration, so its ACT/GpSimd/DVE work doesn't
                        # sit on the iteration boundary and stall the next
                        # score block's PSUM reuse
                        if j == 6 and pending is not None:
                            emit_norm(*pending)
                            pending = None
                    emit_pv(ST - 3)
                    emit_pv(ST - 2)
                    emit_pv(ST - 1)

                    # copy [out|denom] rows out of PSUM immediately so the
                    # accumulator banks free for the next iteration; the rest
                    # of the normalization is deferred
                    uvs = []
                    for half in range(2):
                        uv = smalls.tile([65, 512], F32)
                        nc.vector.tensor_copy(uv[:], pvs[half][0:65, :])
                        uvs.append(uv)
                    pending = (uvs, hp, ich)
            emit_norm(*pending)

        # ---------------- Phase C: out projection ----------------
        with ExitStack() as cctx:
            yt_pool = cctx.enter_context(tc.tile_pool(name="yt", bufs=3))
            y_ps = cctx.enter_context(
                tc.tile_pool(name="y_ps", bufs=2, space="PSUM"))

            for et in range(2):
                woT = woTs[et]
                for it in range(8):
                    y = y_ps.tile([128, 512], F32)
                    for ci, ct in enumerate([7] + list(range(7))):
                        nc.tensor.matmul(
                            y[:],
                            OT[:, ct * MYROWS + it * 128:
                               ct * MYROWS + (it + 1) * 128],
                            woT[:, ct * 512:(ct + 1) * 512],
                            start=(ci == 0), stop=(ci == CT - 1))
                    yt = yt_pool.tile([128, 512], F32)
                    nc.vector.tensor_add(yt[:], y[:],
                                         be_bc[:, et * 512:(et + 1) * 512])
                    nc.sync.dma_start(
                        out[it * 128:(it + 1) * 128,
                            et * 512:(et + 1) * 512], yt[:])

    nc.compile()
    return nc


_cache = {}


def _get_nc():
    if "nc" not in _cache:
        _cache["nc"] = build()
    return _cache["nc"]


def kernel(x_q, W_qkv, b_qkv, W_out, b_out):
    """Core c of 8 handles batch c//2, query rows [(c%2)*1024, +1024).

    The per-core x slice is ROLLED by the core's query-row offset so every
    core's own query rows sit at rows [0, MYROWS) of its slice. Attention is
    permutation-invariant over keys, so the rolled K/V ordering does not
    change the output.
    """
    x_q = np.ascontiguousarray(x_q, dtype=np.float32)
    W_qkv = np.ascontiguousarray(W_qkv, dtype=np.float32)
    b_qkv = np.ascontiguousarray(b_qkv, dtype=np.float32)
    W_out = np.ascontiguousarray(W_out, dtype=np.float32)
    b_out = np.ascontiguousarray(b_out, dtype=np.float32)

    nc = _get_nc()
    in_maps = build_in_maps(x_q, W_qkv, b_qkv, W_out, b_out)
    res = run_bass_kernel_spmd(nc, in_maps, list(range(NCORES)))
    out = np.empty((B, S, C), dtype=np.float32)
    for c in range(NCORES):
        b, half = c // 2, c % 2
        out[b, half * MYROWS:(half + 1) * MYROWS] = res.results[c]["out"]
    return out


def build_in_maps(x_q, W_qkv, b_qkv, W_out, b_out):
    BF = ml_dtypes.bfloat16
    x_q = np.ascontiguousarray(x_q, dtype=np.float32)
    W_qkv = np.asarray(W_qkv, dtype=np.float32)
    b_qkv = np.ascontiguousarray(b_qkv, dtype=np.float32)
    W_out = np.asarray(W_out, dtype=np.float32)
    b_out = np.ascontiguousarray(b_out, dtype=np.float32)
    # wqk[wt, p, ct*128+f] = W_qkv[wt*128+f, ct*128+p]  (Q/K rows only)
    w4 = W_qkv.reshape(24, 128, CT, 128)            # [wt, f, ct, p]
    wqk = np.ascontiguousarray(
        w4[:16].transpose(0, 3, 2, 1).reshape(16, 128, CT * 128).astype(BF))
    # wv[vch, p, ct*512+f] = W_qkv[2C+vch*512+f, ct*128+p]
    wv5 = W_qkv[2 * C:].reshape(2, 512, CT, 128)    # [vch, f, ct, p]
    wv = np.ascontiguousarray(
        wv5.transpose(0, 3, 2, 1).reshape(2, 128, CT * 512).astype(BF))
    # wo[et, p, ct*512+e] = W_out[et*512+e, ct*128+p]
    wo5 = W_out.reshape(2, 512, CT, 128)            # [et, e, ct, p]
    wo = np.ascontiguousarray(
        wo5.transpose(0, 3, 2, 1).reshape(2, 128, CT * 512).astype(BF))
    bq2d = np.ascontiguousarray(b_qkv[:C].reshape(8, 128).T)
    # V bias folded into the output bias (softmax rows sum to 1):
    # out = PV @ W_out.T + (b_v @ W_out.T + b_out)
    beff = np.ascontiguousarray(b_out + W_out @ b_qkv[2 * C:])
    in_maps = []
    for c in range(NCORES):
        b, half = c // 2, c % 2
        xb = x_q[b]
        if half:
            xb = np.roll(xb, -MYROWS, axis=0)
        in_maps.append({
            "xT": np.ascontiguousarray(xb.T.astype(BF)),
            "wqk": wqk,
            "wv": wv,
            "wo": wo,
            "bq2d": bq2d,
            "beff": beff,
        })
    return in_maps


if __name__ == "__main__":
    # smoke test with random inputs
    rng = np.random.default_rng(0)
    x_q = rng.standard_normal((B, S, C), dtype=np.float32)
    s = 1.0 / np.sqrt(C)
    W_qkv = rng.uniform(-s, s, (3 * C, C)).astype(np.float32)
    b_qkv = rng.uniform(-s, s, 3 * C).astype(np.float32)
    W_out = rng.uniform(-s, s, (C, C)).astype(np.float32)
    b_out = rng.uniform(-s, s, C).astype(np.float32)
    got = kernel(x_q=x_q, W_qkv=W_qkv, b_qkv=b_qkv, W_out=W_out, b_out=b_out)
    print("smoke ok", got.shape, float(np.abs(got).max()))


# revision 37
# speedup vs baseline: 1.5471x; 1.0957x over previous
"""Multi-head attention (B=4, S=2048, C=1024, H=16) on 8 TRN2 NeuronCores.

Sharding: data-parallel over batch (4) x query-row split (2); core c handles
batch c//2, query rows [(c%2)*1024, +1024). The host rolls each core's x by
its query-row offset (attention is permutation-invariant over keys) and
passes x^T plus DMA-friendly re-layouts of the weights, all in bfloat16
(matmuls stream at 1 cycle/row in bf16 vs ~2 for fp32; the 2e-2 rel-err
budget easily covers it). No DRAM scratch: Q^T, K^T, V and the attention
output stay resident in SBUF between phases.

  A) QKV projection in bf16. K bias is dropped (softmax is invariant to a
     per-query constant); V bias is folded into the output bias on the host
     (b_eff = b_out + W_out @ b_v, valid because softmax rows sum to 1).
     V lands strided into a [128, st, head, 65] SBUF layout whose 65th lane
     is pre-set to 1.0 - the PV stationary [V_h | 1] then accumulates the
     softmax denominator at out row 64.
  B) Attention per head-pair: transposed scores sc[j,i] = K_h^T(stationary,
     64 PE rows) x Q_h^T(moving); the two heads run row-packed on PE-array
     halves concurrently, into separate per-half PSUM tiles. Head A uses
     2-j-tile PSUM blocks (2 banks) exp'd exactly on the Scalar ACT engine;
     head B uses 1-j-tile blocks (1 bank) exp'd on the Vector engine as a
     one-instruction Schraudolph bf16-exp (u16 = s*EA + EB, bitcast to bf16;
     the ~1.8% RMS sawtooth error cancels its mean in the softmax ratio).
     ACT alone cannot keep up with the PE, so the softmax cost is split
     across both engines. Both halves are double-buffered (2*2 + 2*1 banks)
     plus 2 PV accumulator banks = all 8 PSUM banks, so exp of block t
     overlaps scores of block t+1 with no write-after-read stalls; PV runs
     two j-tiles behind the scores and fills the PE while the exps run.
     Normalization: the PV accumulator is copied out of PSUM immediately
     (freeing the banks), then the tail - 1/denom as exp(-ln d) on ACT
     (the combined ln+exp table is preloaded once so no mid-stream table
     swaps), partition-broadcast on GpSimd, multiply on Vector - is
     deferred into the middle of the NEXT iteration so it never stalls
     the iteration boundary.
  C) Out-projection with O^T stationary; bias (b_eff) added via a
     partition-broadcast tile; the accumulation order puts the
     last-finished OT strip last so Phase C overlaps the final norm tail.

No collectives (a pairwise K/V AllGather dedup was measured a net loss -
see USE_CC); each core writes its own [1024, 1024] fp32 output slice.
"""

from contextlib import ExitStack

import numpy as np
import ml_dtypes

import concourse.mybir as mybir
import concourse.tile as tile
from concourse import bacc
from concourse.bass_utils import run_bass_kernel_spmd

F32 = mybir.dt.float32
BF16 = mybir.dt.bfloat16
U16 = mybir.dt.uint16
AF = mybir.ActivationFunctionType
MUL = mybir.AluOpType.mult
ADD = mybir.AluOpType.add

B, S, C, H, DH = 4, 2048, 1024, 16, 64
NCORES = 8
SCALE = DH ** -0.5  # 0.125
CT = C // 128  # 8 channel tiles
ST = S // 128  # 16 seq (key) tiles
MYROWS = S // 2  # 1024 query rows per core

# Schraudolph bf16-exp: exp(SCALE*s) ~= bitcast_bf16(u16(s*EA + EB)).
# EA = SCALE * 128/ln2; EB = 127*128 - 7.33 (mean-centering) + 0.5 (in case
# the f32->u16 cast truncates; a half-ulp either way is harmless since the
# softmax ratio cancels constant factors).
EXP_EA = SCALE * 128.0 / float(np.log(2.0))
EXP_EB = 16249.17

# Deduplicate the K/V projection across core pairs: each core computes K/V
# only for its own 1024 keys (the x rows it holds at columns 0-1023; the
# per-core roll makes these disjoint within a pair) and the pair exchanges
# blocks with an AllGather. Keys are then used in rank order on both cores -
# attention is permutation-invariant over keys, so no per-core reindexing is
# needed and the SPMD program stays uniform.
# Measured: the AllGather moves 4MB at only ~42GB/s effective in this
# runtime (a ~96us tensor bubble), losing to the ~65us of matmul savings.
# The exchange is correct but a net loss, so it stays disabled.
USE_CC = False




def build():
    nc = bacc.Bacc("TRN2", target_bir_lowering=False, debug=False,
                   num_devices=NCORES)

    # host-prepared layouts (pure data movement on the host):
    #   xT[c, s] = x[s, c]                                   (bf16)
    #   wqk[wt, p, ct*128+f] = W_qkv[wt*128+f, ct*128+p]     (Q/K strips, bf16)
    #   wv[vch, p, ct*512+f] = W_qkv[2C+vch*512+f, ct*128+p] (bf16)
    #   wo[et, p, ct*512+e] = W_out[et*512+e, ct*128+p]      (bf16)
    #   bq2d[p, wt] = b_qkv[wt*128+p] for wt<8               (f32)
    #   beff[e] = b_out[e] + (W_out @ b_v)[e]                (f32)
    xT_in = nc.dram_tensor("xT", [C, S], BF16, kind="ExternalInput").ap()
    wqk_in = nc.dram_tensor("wqk", [16, 128, CT * 128], BF16,
                            kind="ExternalInput").ap()
    wv_in = nc.dram_tensor("wv", [2, 128, CT * 512], BF16,
                           kind="ExternalInput").ap()
    wo_in = nc.dram_tensor("wo", [2, 128, CT * 512], BF16,
                           kind="ExternalInput").ap()
    bq2d = nc.dram_tensor("bq2d", [128, 8], F32, kind="ExternalInput").ap()
    beff = nc.dram_tensor("beff", [C], F32, kind="ExternalInput").ap()
    out = nc.dram_tensor("out", [MYROWS, C], F32, kind="ExternalOutput").ap()
    if USE_CC:
        # staging for the pairwise K/V exchange: cols 0..8191 = K strips
        # (8 x 1024 own keys), cols 8192..16383 = V (8 st x 1024 ch)
        khv_d = nc.dram_tensor("khv_d", [128, 16 * 1024], BF16).ap()
        khv_g = nc.dram_tensor("khv_g", [2, 128, 16 * 1024], BF16).ap()

    with tile.TileContext(nc) as tc, ExitStack() as ctx:
        const = ctx.enter_context(tc.tile_pool(name="const", bufs=1))
        bq_sb = const.tile([128, 8], F32)  # bq_sb[p, wt] = b_q[wt*128+p]
        nc.sync.dma_start(bq_sb[:], bq2d)
        be_sb = const.tile([1, C], F32)
        nc.sync.dma_start(be_sb[:], beff[None, :])
        be_bc = const.tile([128, C], F32)
        nc.gpsimd.partition_broadcast(be_bc[:], be_sb[0:1, :])

        # Pre-load the combined ln+exp activation table (act_func_set_id 6,
        # natural_log_exp_and_others in the cayman act_info.json) so the
        # softmax Exp stream and the 1/denom Ln never force mid-stream
        # ACT_TABLE_LOAD swaps (~1.3us each, 2 per iteration otherwise).
        nc.scalar.add_instruction(mybir.InstLoadActFuncSet(
            name=f"I-{nc.next_id()}", act_func_set_id=6, ins=[], outs=[]))

        persist = ctx.enter_context(tc.tile_pool(name="persist", bufs=1))
        xT = persist.tile([128, CT * S], BF16)  # xT[p, ct*S+s] = x[s, ct*128+p]
        kT = persist.tile([128, 8 * S], BF16)   # kT[p, hp*S+s] = K^T[hp*128+p, s]
        qT = persist.tile([128, 8 * MYROWS], BF16)  # qT[p, hp*1024+i]
        # vt[p, st, h, d] = V[st*128+p, h*64+d] for d<64; 1.0 at d=64
        vt = persist.tile([128, ST * H * 65], BF16)
        vt4 = vt[:].rearrange("p (t h d) -> p t h d", h=H, d=65)
        nc.gpsimd.memset(vt4[:, :, :, 64:65], 1.0)
        OT = persist.tile([128, CT * MYROWS], BF16)  # OT[p, ct*1024+i]

        # ---------------- Phase A: QKV projection ----------------
        with ExitStack() as actx:
            wstrip = actx.enter_context(tc.tile_pool(name="wstrip", bufs=3))
            vw_pool = actx.enter_context(tc.tile_pool(name="vw", bufs=2))
            acc_ps = actx.enter_context(
                tc.tile_pool(name="acc_ps", bufs=4, space="PSUM"))

            # x^T resident: 8 c-tiles of [128, S]
            for ct in range(CT):
                nc.sync.dma_start(xT[:, ct * S:(ct + 1) * S],
                                  xT_in[ct * 128:(ct + 1) * 128, :])

            ksch = 2 if USE_CC else 4  # with CC, only own 1024 keys
            vst = ST // 2 if USE_CC else ST

            # V natural: rhs = W_v^T chunks, output strided into vt (64 of
            # every 65 lanes; lane 64 holds the pre-set ones column)
            vws = []
            for vch in range(2):
                vw = vw_pool.tile([128, CT * 512], BF16)
                nc.sync.dma_start(vw[:], wv_in[vch])
                vws.append(vw)
            for st in range(vst):
                for vch in range(2):
                    acc = acc_ps.tile([128, 512], F32)
                    for ct in range(CT):
                        nc.tensor.matmul(
                            acc[:],
                            xT[:, ct * S + st * 128: ct * S + (st + 1) * 128],
                            vws[vch][:, ct * 512:(ct + 1) * 512],
                            start=(ct == 0), stop=(ct == CT - 1))
                    nc.vector.tensor_copy(
                        vt4[:, st, vch * 8:(vch + 1) * 8, 0:64],
                        acc[:].rearrange("p (h d) -> p h d", d=64))
                if USE_CC:
                    nc.sync.dma_start(
                        khv_d[:, 8192 + st * 1024: 8192 + (st + 1) * 1024]
                        .rearrange("p (h d) -> p h d", d=64),
                        vt4[:, st, :, 0:64])

            # K^T strips (after V so the first matmul only waits on xT+wv).
            # K needs no bias.
            for wt in range(8, 16):
                ws = wstrip.tile([128, CT * 128], BF16)
                nc.sync.dma_start(ws[:], wqk_in[wt])
                for sch in range(ksch):
                    acc = acc_ps.tile([128, 512], F32)
                    for ct in range(CT):
                        nc.tensor.matmul(
                            acc[:],
                            ws[:, ct * 128:(ct + 1) * 128],
                            xT[:, ct * S + sch * 512: ct * S + sch * 512 + 512],
                            start=(ct == 0), stop=(ct == CT - 1))
                    nc.vector.tensor_copy(
                        kT[:, (wt - 8) * S + sch * 512:
                           (wt - 8) * S + (sch + 1) * 512],
                        acc[:])
                if USE_CC:
                    nc.sync.dma_start(
                        khv_d[:, (wt - 8) * 1024:(wt - 7) * 1024],
                        kT[:, (wt - 8) * S:(wt - 8) * S + 1024])

            if USE_CC:
                nc.gpsimd.collective_compute(
                    "AllGather", mybir.AluOpType.bypass,
                    [[0, 1], [2, 3], [4, 5], [6, 7]],
                    ins=[khv_d], outs=[khv_g])

            # Q^T strips (covers the exchange latency when USE_CC)
            for wt in range(8):
                ws = wstrip.tile([128, CT * 128], BF16)
                nc.sync.dma_start(ws[:], wqk_in[wt])
                for sch in range(2):
                    acc = acc_ps.tile([128, 512], F32)
                    for ct in range(CT):
                        nc.tensor.matmul(
                            acc[:],
                            ws[:, ct * 128:(ct + 1) * 128],
                            xT[:, ct * S + sch * 512: ct * S + sch * 512 + 512],
                            start=(ct == 0), stop=(ct == CT - 1))
                    nc.vector.tensor_scalar_add(
                        qT[:, wt * MYROWS + sch * 512:
                           wt * MYROWS + (sch + 1) * 512],
                        acc[:], bq_sb[:, wt:wt + 1])

            if USE_CC:
                # scatter the gathered blocks: keys live in RANK order on
                # every core (rank r's block -> key cols r*1024, st r*8).
                # The own-rank block is rewritten in place with identical
                # data, which keeps the SPMD program uniform.
                for r in range(2):
                    for wt in range(8):
                        nc.sync.dma_start(
                            kT[:, wt * S + r * 1024: wt * S + (r + 1) * 1024],
                            khv_g[r, :, wt * 1024:(wt + 1) * 1024])
                    for stl in range(8):
                        nc.sync.dma_start(
                            vt4[:, r * 8 + stl, :, 0:64],
                            khv_g[r, :, 8192 + stl * 1024:
                                  8192 + (stl + 1) * 1024]
                            .rearrange("p (h d) -> p h d", d=64))

        # Prefetch the out-projection weights during Phase B so Phase C
        # doesn't stall on their DMA.
        woTs = []
        for et in range(2):
            woT = persist.tile([128, CT * 512], BF16)  # [c_p, ct, 512 e]
            nc.sync.dma_start(woT[:], wo_in[et])
            woTs.append(woT)

        # ---------------- Phase B: attention ----------------
        with ExitStack() as bctx:
            pp = bctx.enter_context(tc.tile_pool(name="pp", bufs=5))
            smalls = bctx.enter_context(tc.tile_pool(name="smalls", bufs=6))
            sc_ps = bctx.enter_context(
                tc.tile_pool(name="sc_ps", bufs=2, space="PSUM"))
            pv_ps = bctx.enter_context(
                tc.tile_pool(name="pv_ps", bufs=1, space="PSUM"))

            def emit_norm(uvs, hp, ich):
                # deferred normalization tail (reads SBUF only): 1/denom as
                # exp(-ln d) on ACT (vector.reciprocal is ~3.3us/call and
                # the approx-fast custom op is broken in this runtime),
                # broadcast on GpSimd, multiply on Vector.
                for half in range(2):
                    uv = uvs[half]
                    lnt = smalls.tile([1, 512], F32)
                    nc.scalar.activation(lnt[:], uv[64:65, :], AF.Ln)
                    rec = smalls.tile([1, 512], F32)
                    nc.scalar.activation(rec[:], lnt[:], AF.Exp, scale=-1.0)
                    rb = smalls.tile([64, 512], F32)
                    nc.gpsimd.partition_broadcast(rb[:], rec[0:1, :])
                    nc.vector.tensor_mul(
                        OT[half * 64:half * 64 + 64,
                           hp * MYROWS + ich * 512:
                           hp * MYROWS + (ich + 1) * 512],
                        uv[0:64, :], rb[:])

            pending = None  # (uvs, hp, ich) whose norm tail is deferred
            # hp=7 first: Phase C's accumulation is ordered to END on hp6's
            # OT strip, so C's first matmuls overlap the final norm tail
            for hp in [7] + list(range(7)):  # A = rows 0-63, B = 64-127
                kt = kT[:, hp * S:(hp + 1) * S]
                for ich in range(2):
                    qt = qT[:, hp * MYROWS + ich * 512:
                            hp * MYROWS + (ich + 1) * 512]
                    pvs = [pv_ps.tile([128, 512], F32, tag=f"pv{half}",
                                      name=f"pv{half}")
                           for half in range(2)]
                    # pg[j] = (ap of exp'd scores for j, per half)
                    pgj = [[None] * ST, [None] * ST]

                    def emit_pv(j):
                        for half in range(2):
                            nc.tensor.matmul(
                                pvs[half][0:65, :],
                                vt4[:, j, 2 * hp + half, 0:65],
                                pgj[half][j],
                                start=(j == 0), stop=(j == ST - 1))

                    sca = None
                    for j in range(ST):
                        jj = j % 2
                        if jj == 0:
                            sca = sc_ps.tile([128, 1024], F32, tag="scA",
                                             name="scA")
                        scb = sc_ps.tile([128, 512], F32, tag="scB",
                                         name="scB")
                        # row-disjoint pair runs concurrently on the PE array
                        nc.tensor.matmul(sca[:, jj * 512:(jj + 1) * 512],
                                         kt[0:64, j * 128:(j + 1) * 128],
                                         qt[0:64, :], start=True, stop=True)
                        nc.tensor.matmul(scb[:],
                                         kt[64:128, j * 128:(j + 1) * 128],
                                         qt[64:128, :], start=True, stop=True)
                        # half B: Schraudolph bf16-exp on the Vector engine
                        # (last two j on ACT instead - DVE is the busier
                        # engine, and this also trims the approx fraction)
                        pgb = pp.tile([128, 512], BF16, tag="pgb", name="pgb")
                        if j < 14:
                            nc.vector.tensor_scalar(
                                pgb[:].bitcast(U16), scb[:],
                                EXP_EA, EXP_EB, MUL, ADD)
                        else:
                            nc.scalar.activation(pgb[:], scb[:],
                                                 AF.Exp, scale=SCALE)
                        pgj[1][j] = pgb[:]
                        # half A: exact exp on the ACT engine, per 2-j block
                        if jj == 1:
                            pga = pp.tile([128, 1024], BF16, tag="pga",
                                          name="pga")
                            nc.scalar.activation(pga[:], sca[:],
                                                 AF.Exp, scale=SCALE)
                            pgj[0][j - 1] = pga[:, 0:512]
                            pgj[0][j] = pga[:, 512:1024]
                        # software pipeline: PV trails the scores by 3 j so
                        # PV_A never waits on the 2-j-block ACT exp (~1.1us,
                        # about the tensor work per 2 j - a 2-j trail left
                        # the PE stalling a few hundred ns per block)
                        if j >= 3:
                            emit_pv(j - 3)
                        # previous iteration's normalization tail runs here,
                        # mid-iteration, so its ACT/GpSimd/DVE work doesn't
                        # sit on the iteration boundary and stall the next
                        # score block's PSUM reuse
                        if j == 6 and pending is not None:
                            emit_norm(*pending)
                            pending = None
                    emit_pv(ST - 3)
                    emit_pv(ST - 2)
                    emit_pv(ST - 1)

                    # copy [out|denom] rows out of PSUM immediately so the
                    # accumulator banks free for the next iteration; the rest
                    # of the normalization is deferred
                    uvs = []
                    for half in range(2):
                        uv = smalls.tile([65, 512], F32)
                        nc.vector.tensor_copy(uv[:], pvs[half][0:65, :])
                        uvs.append(uv)
                    pending = (uvs, hp, ich)
            emit_norm(*pending)

        # ---------------- Phase C: out projection ----------------
        with ExitStack() as cctx:
            yt_pool = cctx.enter_context(tc.tile_pool(name="yt", bufs=3))
            y_ps = cctx.enter_context(
                tc.tile_pool(name="y_ps", bufs=2, space="PSUM"))

            for et in range(2):
                woT = woTs[et]
                for it in range(8):
                    y = y_ps.tile([128, 512], F32)
                    for ci, ct in enumerate([7] + list(range(7))):
                        nc.tensor.matmul(
                            y[:],
                            OT[:, ct * MYROWS + it * 128:
                               ct * MYROWS + (it + 1) * 128],
                            woT[:, ct * 512:(ct + 1) * 512],
                            start=(ci == 0), stop=(ci == CT - 1))
                    yt = yt_pool.tile([128, 512], F32)
                    nc.vector.tensor_add(yt[:], y[:],
                                         be_bc[:, et * 512:(et + 1) * 512])
                    nc.sync.dma_start(
                        out[it * 128:(it + 1) * 128,
                            et * 512:(et + 1) * 512], yt[:])

    nc.compile()
    return nc


_cache = {}


def _get_nc():
    if "nc" not in _cache:
        _cache["nc"] = build()
    return _cache["nc"]


def kernel(x_q, W_qkv, b_qkv, W_out, b_out):
    """Core c of 8 handles batch c//2, query rows [(c%2)*1024, +1024).

    The per-core x slice is ROLLED by the core's query-row offset so every
    core's own query rows sit at rows [0, MYROWS) of its slice. Attention is
    permutation-invariant over keys, so the rolled K/V ordering does not
    change the output.
    """
    x_q = np.ascontiguousarray(x_q, dtype=np.float32)
    W_qkv = np.ascontiguousarray(W_qkv, dtype=np.float32)
    b_qkv = np.ascontiguousarray(b_qkv, dtype=np.float32)
    W_out = np.ascontiguousarray(W_out, dtype=np.float32)
    b_out = np.ascontiguousarray(b_out, dtype=np.float32)

    nc = _get_nc()
    in_maps = build_in_maps(x_q, W_qkv, b_qkv, W_out, b_out)
    res = run_bass_kernel_spmd(nc, in_maps, list(range(NCORES)))
    out = np.empty((B, S, C), dtype=np.float32)
    for c in range(NCORES):
        b, half = c // 2, c % 2
        out[b, half * MYROWS:(half + 1) * MYROWS] = res.results[c]["out"]
    return out


def build_in_maps(x_q, W_qkv, b_qkv, W_out, b_out):
    BF = ml_dtypes.bfloat16
    x_q = np.ascontiguousarray(x_q, dtype=np.float32)
    W_qkv = np.asarray(W_qkv, dtype=np.float32)
    b_qkv = np.ascontiguousarray(b_qkv, dtype=np.float32)
    W_out = np.asarray(W_out, dtype=np.float32)
    b_out = np.ascontiguousarray(b_out, dtype=np.float32)
    # wqk[wt, p, ct*128+f] = W_qkv[wt*128+f, ct*128+p]  (Q/K rows only)
    w4 = W_qkv.reshape(24, 128, CT, 128)            # [wt, f, ct, p]
    wqk = np.ascontiguousarray(
        w4[:16].transpose(0, 3, 2, 1).reshape(16, 128, CT * 128).astype(BF))
    # wv[vch, p, ct*512+f] = W_qkv[2C+vch*512+f, ct*128+p]
    wv5 = W_qkv[2 * C:].reshape(2, 512, CT, 128)    # [vch, f, ct, p]
    wv = np.ascontiguousarray(
        wv5.transpose(0, 3, 2, 1).reshape(2, 128, CT * 512).astype(BF))
    # wo[et, p, ct*512+e] = W_out[et*512+e, ct*128+p]
    wo5 = W_out.reshape(2, 512, CT, 128)            # [et, e, ct, p]
    wo = np.ascontiguousarray(
        wo5.transpose(0, 3, 2, 1).reshape(2, 128, CT * 512).astype(BF))
    bq2d = np.ascontiguousarray(b_qkv[:C].reshape(8, 128).T)
    # V bias folded into the output bias (softmax rows sum to 1):
    # out = PV @ W_out.T + (b_v @ W_out.T + b_out)
    beff = np.ascontiguousarray(b_out + W_out @ b_qkv[2 * C:])
    in_maps = []
    for c in range(NCORES):
        b, half = c // 2, c % 2
        xb = x_q[b]
        if half:
            xb = np.roll(xb, -MYROWS, axis=0)
        in_maps.append({
            "xT": np.ascontiguousarray(xb.T.astype(BF)),
            "wqk": wqk,
            "wv": wv,
            "wo": wo,
            "bq2d": bq2d,
            "beff": beff,
        })
    return in_maps


if __name__ == "__main__":
    # smoke test with random inputs
    rng = np.random.default_rng(0)
    x_q = rng.standard_normal((B, S, C), dtype=np.float32)
    s = 1.0 / np.sqrt(C)
    W_qkv = rng.uniform(-s, s, (3 * C, C)).astype(np.float32)
    b_qkv = rng.uniform(-s, s, 3 * C).astype(np.float32)
    W_out = rng.uniform(-s, s, (C, C)).astype(np.float32)
    b_out = rng.uniform(-s, s, C).astype(np.float32)
    got = kernel(x_q=x_q, W_qkv=W_qkv, b_qkv=b_qkv, W_out=W_out, b_out=b_out)
    print("smoke ok", got.shape, float(np.abs(got).max()))
